# revision 1
# baseline (speedup 1.0000x reference)
import numpy as np
import jax
import jax.numpy as jnp
from functools import partial

# Problem constants (nn_GaussianEmbedding): hardcoded per harness contract.
NUM_TERMS = 8      # h has NUM_TERMS+1 = 9 rows
NUM_CHANNELS = 64
BATCH = 8
NODES = 2048


def _diags_one(A):
    # A: [N, N] -> D: [T, N] where D[i] = diag(A^(i+1))
    diags = []
    Ap = A
    for i in range(NUM_TERMS):
        if i > 0:
            Ap = jnp.matmul(Ap, A)
        diags.append(jnp.diagonal(Ap))
    return jnp.stack(diags, axis=0)  # [T, N]


def _per_core(A_b, h):
    # A_b: [N, N] (one batch element per core), h: [T+1, C]
    D = _diags_one(A_b)                       # [T, N]
    out = jnp.einsum("tn,tc->nc", D, h[1:])   # [N, C]
    return out + h[0][None, :]


_pmapped = None


def _get_pmapped():
    global _pmapped
    if _pmapped is None:
        _pmapped = jax.pmap(_per_core, in_axes=(0, None))
    return _pmapped


def kernel(A: np.ndarray, h: np.ndarray) -> np.ndarray:
    A = np.asarray(A, dtype=np.float32)
    h = np.asarray(h, dtype=np.float32)
    n_dev = jax.local_device_count()
    if n_dev >= BATCH:
        # Data-parallel over batch: one 2048x2048 chain of 7 matmuls per core.
        out = _get_pmapped()(A, h)            # [B, N, C]
        return np.asarray(out, dtype=np.float32)
    # Fallback: single-device vmap (correctness path).
    out = jax.vmap(_per_core, in_axes=(0, None))(jnp.asarray(A), jnp.asarray(h))
    return np.asarray(out, dtype=np.float32)



# revision 36
# speedup vs baseline: 2838.0271x; 2838.0271x over previous
"""Trainium2 Bass kernel for nn_GaussianEmbedding.

Computes Y[b,n,c] = h[0,c] + sum_{i=1..8} h[i,c] * diag(A^i)[b,n]
for A: [8, 2048, 2048] f32, h: [9, 64] f32.

Strategy (per core, one batch element per NeuronCore, 8 cores):
  Only diagonals of A^1..A^8 are needed, so only 3 full 2048^3 matmuls
  run (bf16, f32 accumulate), against the reference's 7 in f32:
      A2 = mm(lhsT=At, rhs=A)  = A^2        (row-major)
      B3 = mm(lhsT=A2, rhs=At) = (A^3)^T    (col-major)
      A5 = mm(lhsT=B3, rhs=A2) = A^5        (PSUM only, never stored)
  Diagonals: diag(A^(p+q))[n] = sum_k P[n,k] Q[k,n], evaluated as fused
  multiply+rowsum (affine_mul_reduce) on the DVE:
      d1 = diag(A), d2 = diag(A2), d3 = diag(B3), d5 = diag(A5)
      d6 = rowsum(A5 (.) At), d8 = rowsum(A5 (.) B3)   [direct, At/B3
          are the transposed factors already resident in SBUF]
      d4, d7 = rowsum({A2, A5} (.) A2^T-tiles)          [one PE sweep]
  All dot work pipelines per output strip behind the matmuls (~17us
  total on HW).  Y = [1, d1..d8] @ h via a tiny f32 PE matmul per strip.

bf16 matmul chain + f32 accumulation gives rel err ~6e-3 on the full
output (validated against an f64 reference), inside the 2e-2 gate.
"""

import numpy as np

import concourse.bass as bass
import concourse.tile as tile
from concourse import bacc, mybir
from concourse.masks import make_identity

F32 = mybir.dt.float32
BF16 = mybir.dt.bfloat16

# Problem constants (hardcoded per harness contract).
NUM_TERMS = 8
C = 64          # channels
BATCH = 8
N_FULL = 2048
P = 128         # partitions

# Variant switches (for experiments; defaults = fastest measured config).
USE_DMA_AT = False      # build At with DMA xbar transposes (else PE)
MM_ONLY = False          # timing experiment: matmul chain only, no dots


def _build_body(nc, tc, ctx, a, h, y, N, alt_state):
    """Emit one evaluation of the kernel body for an [N, N] input."""
    S = N // P                   # row strips
    CH = min(1024, N)            # matmul psum width
    NCH = N // CH
    HS = S // 2                  # strips per staging buffer half
    T1 = NUM_TERMS + 1

    pools = alt_state["pools"]
    big = pools["big"]
    scr = pools["scr"]
    small = pools["small"]
    psmm = pools["psmm"]
    pssw = pools.get("pssw")
    psy = pools["psy"]
    psyb = pools["psyb"]
    consts = alt_state["consts"]
    I_bf = consts["I_bf"]
    I_f32 = consts["I_f32"]
    H_sb = consts["H_sb"]

    def alt():
        """Round-robin copy op between DVE (tensor_copy) and ACT (copy)."""
        alt_state["i"] += 1
        if alt_state["i"] & 1:
            return lambda out, in_: nc.vector.tensor_copy(out, in_)
        return lambda out, in_: nc.scalar.copy(out, in_)

    def dma_eng():
        """Round-robin HWDGE queue for xbar transposes."""
        alt_state["d"] += 1
        return nc.sync if (alt_state["d"] & 1) else nc.scalar

    # Big matrices, strip-major: store[:, s, :] = M[128*s:128*(s+1), :].
    # Slot plan (3 x 64KB/partition):
    #   m_at: At (whole kernel)
    #   m0:   A_bf  -> A^3
    #   m1:   f32 staging (two half-tiles) -> A^2 -> A^4
    At = big.tile([P, S * N], BF16, tag="m_at", name="At")
    A_bf = big.tile([P, S * N], BF16, tag="m0", name="A_bf")
    At3 = At.rearrange("p (s n) -> p s n", s=S)
    A3 = A_bf.rearrange("p (s n) -> p s n", s=S)

    # Diagonal accumulator: Dp[:, s, t]; t=0 is the ones column.
    Dp = small.tile([P, S * T1], F32, tag="dp", bufs=1, name="Dp")
    Dp3 = Dp.rearrange("p (s t) -> p s t", s=S)
    nc.gpsimd.memset(Dp3[:, :, 0:1], 1.0)

    # ---- Stage 0: load A (f32) into the m1 slot, cast to bf16,
    #      extract d1, build At via DMA xbar transposes ----
    for half in range(2):
        stg = big.tile([P, HS * N], F32, tag="m1", name=f"stg{half}")
        stg3 = stg.rearrange("p (s n) -> p s n", s=HS)
        for si in range(HS):
            i = half * HS + si
            dma_eng().dma_start(stg3[:, si, :], a.ap()[i * P:(i + 1) * P, :])
            alt()(A3[:, i, :], stg3[:, si, :])
            if MM_ONLY:
                d1scr = None
            else:
                d1scr = scr.tile([P, P], BF16, tag="scrb", name="d1scr")
            if d1scr is not None:
                nc.vector.affine_mul_reduce(
                out=d1scr[:], accum_out=Dp3[:, i, 1:2],
                in0=stg3[:, si, i * P:(i + 1) * P], in1=I_f32[:],
                scale=1.0, bias=0.0)
            # At[:, :, 128i:128(i+1)] = transpose of A strip i
            if USE_DMA_AT:
                dma_eng().dma_start_transpose(
                    out=At3[:, :, i * P:(i + 1) * P], in_=A3[:, i, :])
            else:
                for g in range(S // 4):
                    ps = pssw.tile([P, 512], BF16, tag="aux", name="ps_at")
                    for jj in range(4):
                        j = 4 * g + jj
                        nc.tensor.transpose(ps[:, jj * P:(jj + 1) * P],
                                            A3[:, i, j * P:(j + 1) * P], I_bf[:])
                    out_view = At3[:, 4 * g:4 * g + 4, i * P:(i + 1) * P]
                    alt()(out_view, ps.rearrange("p (j q) -> p j q", j=4))

    # ---- Matmul chain stages ----
    # matmul computes lhsT.T @ rhs; with strip-major storage, using stage
    # output X directly as the next lhsT multiplies by X^T on the left:
    #   A2 = mm(lhsT=At, rhs=A)    = A @ A
    #   B3 = mm(lhsT=A2, rhs=At)   = (A2)^T A^T = (A @ A2)^T = (A^3)^T
    #   A5 = mm(lhsT=B3, rhs=A2)   = A^3 @ A^2
    def matmul_stage(lhsT3, prev3, out_tag, out_name, dslot, per_strip=None):
        Pw = big.tile([P, S * N], BF16, tag=out_tag, name=out_name)
        Pw3 = Pw.rearrange("p (s n) -> p s n", s=S)
        for i in range(S):
            # k-major over the whole strip: each stationary tile is loaded
            # once per (i, k) and feeds all N output columns.
            pss = [psmm.tile([P, CH], F32, tag="mm", bufs=3, name="ps_mm")
                   for _ in range(NCH)]
            for k in range(S):
                lhsT = lhsT3[:, k, i * P:(i + 1) * P]
                for hh in range(NCH):
                    for nb in range(CH // 512):
                        nc.tensor.matmul(
                            pss[hh][:, nb * 512:(nb + 1) * 512], lhsT,
                            prev3[:, k, hh * CH + nb * 512:hh * CH + (nb + 1) * 512],
                            start=(k == 0), stop=(k == S - 1))
            for hh in range(NCH):
                alt()(Pw3[:, i, hh * CH:(hh + 1) * CH], pss[hh][:])
            if not MM_ONLY:
                # diagonal extraction for this strip
                dscr = scr.tile([P, CH], BF16, tag="scrb", name="dscr")
                nc.vector.affine_mul_reduce(
                    out=dscr[:, 0:P], accum_out=Dp3[:, i, dslot:dslot + 1],
                    in0=Pw3[:, i, i * P:(i + 1) * P], in1=I_bf[:],
                    scale=1.0, bias=0.0)
            if per_strip is not None:
                per_strip(Pw3, i)
        return Pw3

    # 4-tile (512-wide) transpose groups per strip
    G = max(1, S // 4)

    def emit_y(i):
        psT = psy.tile([T1, P], F32, tag="aux", name="psT")
        nc.tensor.transpose(psT[:], Dp3[:, i, 0:T1], I_f32[:])
        DpT = small.tile([T1, P], F32, tag="dpt", bufs=2, name="DpT")
        nc.scalar.copy(DpT[:], psT[:])
        psY = psyb.tile([P, C], F32, tag="aux", name="psY")
        nc.tensor.matmul(psY[:], DpT[:], H_sb[:], start=True, stop=True)
        Ysb = small.tile([P, C], F32, tag="ysb", bufs=2, name="Ysb")
        nc.scalar.copy(Ysb[:], psY[:])
        nc.sync.dma_start(y.ap()[i * P:(i + 1) * P, :], Ysb[:])

    # Chain: A2 (row-major), B3 = (A^3)^T (col-major), A5 = A^3 @ A^2
    # (PSUM-resident only, never stored).  Diagonals:
    #   d2 = diag(A2), d3 = diag(B3) (= diag(A^3))
    #   per A5 psum strip i (consumed in place, f32):
    #     d5 = diag(A5), d6 = rowsum(A5 (.) At)  [At[n,k] = A[k,n]]
    #     d8 = rowsum(A5 (.) B3), d7 = rowsum(A5 (.) A2^T-tiles)
    #   d4 from the same PE sweep of A2 that serves d7.
    # Everything pipelines per strip -> negligible post-matmul tail.
    A2_3 = matmul_stage(At3, A3, "m1", "A2", 2)
    B3_3 = matmul_stage(A2_3, At3, "m0", "B3", 3)

    # ---- Stage 3: A5 = A^3 @ A^2 directly out of PSUM ----
    for i in range(S):
        prt5 = small.tile([P, 1], F32, tag="prt", bufs=4, name="prt5")
        prt6 = small.tile([P, NCH], F32, tag="prt", bufs=4, name="prt6")
        prt8 = small.tile([P, NCH], F32, tag="prt", bufs=4, name="prt8")
        prt4 = small.tile([P, G], F32, tag="prtg", bufs=4, name="prt4")
        prt7 = small.tile([P, G], F32, tag="prtg", bufs=4, name="prt7")
        pss5 = [psmm.tile([P, CH], F32, tag="mm", bufs=3, name="ps_a5")
                for _ in range(NCH)]
        for k in range(S):
            lhsT = B3_3[:, k, i * P:(i + 1) * P]
            for hh in range(NCH):
                for nb in range(CH // 512):
                    nc.tensor.matmul(
                        pss5[hh][:, nb * 512:(nb + 1) * 512], lhsT,
                        A2_3[:, k, hh * CH + nb * 512:hh * CH + (nb + 1) * 512],
                        start=(k == 0), stop=(k == S - 1))
        if MM_ONLY:
            # drain psum minimally so the tiles have readers
            mscr = scr.tile([P, CH], BF16, tag="scrb", name="mscr")
            for hh in range(NCH):
                nc.vector.tensor_copy(mscr[:, 0:P], pss5[hh][:, 0:P])
            emit_y(i)
            continue
        for hh in range(NCH):
            ps = pss5[hh]
            # d5: diagonal block of A5 lives in chunk hh == i*P // CH
            if hh == (i * P) // CH:
                off = i * P - hh * CH
                dscr = scr.tile([P, CH], BF16, tag="scrb", name="d5scr")
                nc.vector.affine_mul_reduce(
                    out=dscr[:, 0:P], accum_out=prt5[:, 0:1],
                    in0=ps[:, off:off + P], in1=I_bf[:], scale=1.0, bias=0.0)
            # d6 += rowsum(A5_chunk (.) At_chunk); d8 += (.) B3_chunk
            for (src3, prt) in ((At3, prt6), (B3_3, prt8)):
                sscr = scr.tile([P, CH], BF16, tag="scrb", name="sscr")
                nc.vector.affine_mul_reduce(
                    out=sscr[:], accum_out=prt[:, hh:hh + 1],
                    in0=ps[:], in1=src3[:, i, hh * CH:(hh + 1) * CH],
                    scale=1.0, bias=0.0)
            # d7 += rowsum(A5_group (.) A2^T-tiles); d4 from the same tiles.
            # A2^T group is staged PSUM->SBUF once (DVE reads at most one
            # PSUM operand per instruction) and serves both dots.
            for gg in range(CH // 512):
                g = hh * (CH // 512) + gg
                pst = pssw.tile([P, 512], BF16, tag="aux", name="ps_sw")
                for jj in range(4):
                    j = 4 * g + jj
                    nc.tensor.transpose(pst[:, jj * P:(jj + 1) * P],
                                        A2_3[:, j, i * P:(i + 1) * P], I_bf[:])
                a2t = scr.tile([P, 512], BF16, tag="a2t", name="a2t")
                alt()(a2t[:], pst[:])
                s4cr = scr.tile([P, CH], BF16, tag="scrb", name="s4cr")
                nc.vector.affine_mul_reduce(
                    out=s4cr[:, 0:512], accum_out=prt4[:, g:g + 1],
                    in0=A2_3[:, i, g * 512:(g + 1) * 512], in1=a2t[:],
                    scale=1.0, bias=0.0)
                s7cr = scr.tile([P, CH], BF16, tag="scrb", name="s7cr")
                nc.vector.affine_mul_reduce(
                    out=s7cr[:, 0:512], accum_out=prt7[:, g:g + 1],
                    in0=ps[:, gg * 512:(gg + 1) * 512], in1=a2t[:],
                    scale=1.0, bias=0.0)
        # fold partials into Dp
        nc.vector.tensor_copy(Dp3[:, i, 5:6], prt5[:])
        for (prt, dslot) in ((prt6, 6), (prt8, 8)):
            if NCH > 1:
                nc.vector.tensor_reduce(Dp3[:, i, dslot:dslot + 1], prt[:],
                                        axis=mybir.AxisListType.X,
                                        op=mybir.AluOpType.add)
            else:
                nc.vector.tensor_copy(Dp3[:, i, dslot:dslot + 1], prt[:])
        for (prt, dslot) in ((prt4, 4), (prt7, 7)):
            if G > 1:
                nc.vector.tensor_reduce(Dp3[:, i, dslot:dslot + 1], prt[:],
                                        axis=mybir.AxisListType.X,
                                        op=mybir.AluOpType.add)
            else:
                nc.vector.tensor_copy(Dp3[:, i, dslot:dslot + 1], prt[:])
        emit_y(i)


def build_nc(N=N_FULL, repeat=1, loop_repeat=1):
    """Build and compile the Bacc module. Returns nc.

    repeat: python-unrolled body repetitions (code size scales).
    loop_repeat: hardware For_i loop around the body (code size 1x);
        used only for timing measurements.
    """
    nc = bacc.Bacc("TRN2", target_bir_lowering=False, debug=False,
                   num_devices=BATCH)
    a = nc.dram_tensor("a", [N, N], F32, kind="ExternalInput")
    h = nc.dram_tensor("h", [NUM_TERMS + 1, C], F32, kind="ExternalInput")
    y = nc.dram_tensor("y", [N, C], F32, kind="ExternalOutput")

    with tile.TileContext(nc) as tc:
        import contextlib
        with contextlib.ExitStack() as ctx:
            pools = {
                "big": ctx.enter_context(tc.tile_pool(name="big", bufs=1)),
                "scr": ctx.enter_context(tc.tile_pool(name="scr", bufs=2)),
                "small": ctx.enter_context(tc.tile_pool(name="small", bufs=1)),
            }
            pspool = ctx.enter_context(
                tc.tile_pool(name="ps", bufs=2, space="PSUM"))
            pools["psmm"] = pspool
            pools["pssw"] = pspool
            pools["psy"] = pspool
            pools["psyb"] = pspool
            consts = {}
            cp = ctx.enter_context(tc.tile_pool(name="consts", bufs=1))
            I_bf = cp.tile([P, P], BF16, name="I_bf")
            make_identity(nc, I_bf)
            I_f32 = cp.tile([P, P], F32, name="I_f32")
            make_identity(nc, I_f32)
            H_sb = cp.tile([NUM_TERMS + 1, C], F32, name="H_sb")
            nc.sync.dma_start(H_sb[:], h.ap()[:, :])
            consts["I_bf"] = I_bf
            consts["I_f32"] = I_f32
            consts["H_sb"] = H_sb

            alt_state = {"i": 0, "d": 0, "pools": pools, "consts": consts}
            if loop_repeat > 1:
                with tc.For_i(0, loop_repeat, 1):
                    _build_body(nc, tc, ctx, a, h, y, N, alt_state)
            else:
                for _ in range(repeat):
                    _build_body(nc, tc, ctx, a, h, y, N, alt_state)

    nc.compile()
    return nc


# ---------------- Runner (cached jit over the 8 axon cores) ----------------

_RUNNER = None


def _make_runner(nc, n_cores):
    import jax
    from jax.sharding import Mesh, PartitionSpec, NamedSharding
    from jax.experimental.shard_map import shard_map
    from concourse.bass2jax import (_bass_exec_p, install_neuronx_cc_hook,
                                    partition_id_tensor)

    install_neuronx_cc_hook()

    partition_name = nc.partition_id_tensor.name if nc.partition_id_tensor else None
    in_names, out_names, out_avals, zero_outs = [], [], [], []
    for alloc in nc.m.functions[0].allocations:
        if not isinstance(alloc, mybir.MemoryLocationSet):
            continue
        name = alloc.memorylocations[0].name
        if alloc.kind == "ExternalInput":
            if name != partition_name:
                in_names.append(name)
        elif alloc.kind == "ExternalOutput":
            shape = tuple(alloc.tensor_shape)
            dtype = mybir.dt.np(alloc.dtype)
            out_avals.append(jax.core.ShapedArray(shape, dtype))
            out_names.append(name)
            zero_outs.append(np.zeros((n_cores * shape[0], *shape[1:]), dtype))
    n_params = len(in_names)
    n_outs = len(out_names)
    in_names_all = list(in_names) + out_names
    if partition_name is not None:
        in_names_all.append(partition_name)

    def _body(*args):
        operands = list(args)
        if partition_name is not None:
            operands.append(partition_id_tensor())
        outs = _bass_exec_p.bind(
            *operands,
            out_avals=tuple(out_avals),
            in_names=tuple(in_names_all),
            out_names=tuple(out_names),
            lowering_input_output_aliases=(),
            sim_require_finite=True,
            sim_require_nnan=True,
            nc=nc,
        )
        return tuple(outs)

    devices = jax.devices()[:n_cores]
    mesh = Mesh(np.asarray(devices), ("core",))
    in_specs = (PartitionSpec("core"),) * (n_params + n_outs)
    out_specs = (PartitionSpec("core"),) * n_outs
    sharded = jax.jit(
        shard_map(_body, mesh=mesh, in_specs=in_specs, out_specs=out_specs,
                  check_rep=False),
        keep_unused=True,
    )
    shard = NamedSharding(mesh, PartitionSpec("core"))

    def run(in_maps):
        concat_in = [
            np.concatenate([np.asarray(m[name]) for m in in_maps], axis=0)
            for name in in_names
        ]
        args = [jax.device_put(x, shard) for x in concat_in]
        args += [jax.device_put(z, shard) for z in zero_outs]
        outs = sharded(*args)
        outs = [np.asarray(o) for o in outs]
        per_core = []
        for cidx in range(n_cores):
            d = {}
            for oi, name in enumerate(out_names):
                s0 = out_avals[oi].shape[0]
                d[name] = outs[oi][cidx * s0:(cidx + 1) * s0]
            per_core.append(d)
        return per_core

    run.sharded = sharded
    run.shard = shard
    run.in_names = in_names
    run.out_names = out_names
    run.out_avals = out_avals
    run.zero_outs = zero_outs
    return run


def _get_runner():
    global _RUNNER
    if _RUNNER is None:
        nc = build_nc(N_FULL, repeat=1)
        _RUNNER = _make_runner(nc, BATCH)
    return _RUNNER


def kernel(A: np.ndarray, h: np.ndarray) -> np.ndarray:
    A = np.ascontiguousarray(np.asarray(A, dtype=np.float32))
    h = np.ascontiguousarray(np.asarray(h, dtype=np.float32))
    run = _get_runner()
    in_maps = [{"a": A[b], "h": h} for b in range(BATCH)]
    res = run(in_maps)
    out = np.stack([res[b]["y"] for b in range(BATCH)], axis=0)
    return out.astype(np.float32)
